# revision 1
# baseline (speedup 1.0000x reference)
"""DifferentiableRaster Trainium2 Bass kernel (v3).
Contract: kernel(point_clouds=[32,262144,3] f32) -> [32,1024,1024] f32.
Sharding: data-parallel over batch across 8 NeuronCores (4 batches/core), no
cross-core communication.

Data-parallel over batch: 8 cores x 4 batches. Per batch:
  dist = (z-zmin)/(zmax-zmin)   -- correctly-rounded division via Dekker+Newton
  q01  = exact 1% linear-interp quantile of dist via branchless bisection
  w    = 1 - max(dist, q01)
  idx  = ((x-xmin)/(xmax-xmin))*1022 + 1  -- Dekker division; floor via RNE cast+fix
  bilinear scatter-add into [1024,1024] via per-chunk one-hot matmuls (fp16
  one-hots exact; weights fp16 ~5e-4 rel; accumulate in f32 PSUM).
"""
import sys

for _p in ("/opt/trn_rl_repo", "/root/.axon_site/_ro/trn_rl_repo"):
    if _p not in sys.path:
        sys.path.insert(0, _p)

import numpy as np

try:
    import concourse.bass as bass
    import concourse.bacc as bacc
    import concourse.mybir as mybir
    import concourse.tile as tile
    from concourse import bass_utils
    from concourse.masks import make_identity
except ImportError:  # environments exposing concourse modules top-level
    import bass, bacc, mybir, tile, bass_utils
    from masks import make_identity

F32 = mybir.dt.float32
F16 = mybir.dt.float16
I32 = mybir.dt.int32
U8 = mybir.dt.uint8
OP = mybir.AluOpType
AX = mybir.AxisListType

H = W = 1024
NB = 4
NPT = 262144
CPP = NPT // 128
QRANK = 2621
QFRAC = float(np.float32(np.float32(0.01) * (NPT - 1)) - QRANK)
N_BISECT = 26
SPLIT_C = 4097.0   # Dekker split constant for f32


def _cross(nc, sb, ps, val_p, op, ident, tag):
    tp = ps.tile([128, 128], F32, tag="xpose", space="PSUM", name="tp")
    nc.tensor.transpose(tp[:1, :128], val_p[:, :1], ident[:])
    row = sb.tile([1, 128], F32, tag="xrow", name="row")
    nc.vector.tensor_copy(row[:], tp[:1, :128])
    out = sb.tile([1, 1], F32, tag=tag, name="out")
    nc.vector.tensor_reduce(out[:], row[:], axis=AX.X, op=op)
    return out


def _bcast(nc, sb, scalar, tag):
    out = sb.tile([128, 1], F32, tag=tag, name="out")
    nc.gpsimd.partition_broadcast(out[:], scalar[:])
    return out


def _scalar_prep(nc, sb, ps, Vv, ident, tag):
    """min/max/span/recip + Dekker split of span, all broadcast. Returns dict."""
    rmin = sb.tile([128, 1], F32, tag="rmin", name="rmin")
    rmax = sb.tile([128, 1], F32, tag="rmax", name="rmax")
    nc.vector.tensor_reduce(rmin[:], Vv, axis=AX.X, op=OP.min)
    nc.vector.tensor_reduce(rmax[:], Vv, axis=AX.X, op=OP.max)
    gmin = _cross(nc, sb, ps, rmin, OP.min, ident, f"gmin{tag}")
    gmax = _cross(nc, sb, ps, rmax, OP.max, ident, f"gmax{tag}")
    span = sb.tile([1, 1], F32, tag="span", name="span")
    nc.vector.tensor_tensor(out=span[:], in0=gmax[:], in1=gmin[:], op=OP.subtract)
    rsp = sb.tile([1, 1], F32, tag="rsp", name="rsp")
    nc.vector.reciprocal(rsp[:], span[:])
    # Dekker split of span: mhi + mlo == span exactly
    t = sb.tile([1, 1], F32, tag="dk_t", name="t")
    u = sb.tile([1, 1], F32, tag="dk_u", name="u")
    mhi = sb.tile([1, 1], F32, tag="dk_hi", name="mhi")
    mlo = sb.tile([1, 1], F32, tag="dk_lo", name="mlo")
    nc.vector.tensor_scalar(t[:], span[:], SPLIT_C, None, OP.mult)
    nc.vector.tensor_tensor(out=u[:], in0=t[:], in1=span[:], op=OP.subtract)
    nc.vector.tensor_tensor(out=mhi[:], in0=t[:], in1=u[:], op=OP.subtract)
    nc.vector.tensor_tensor(out=mlo[:], in0=span[:], in1=mhi[:], op=OP.subtract)
    return {
        "minb": _bcast(nc, sb, gmin, "minb"),
        "spanb": _bcast(nc, sb, span, "spanb"),
        "rspb": _bcast(nc, sb, rsp, "rspb"),
        "mhib": _bcast(nc, sb, mhi, "mhib"),
        "mlob": _bcast(nc, sb, mlo, "mlob"),
    }


def _dekker_div(nc, OUT, T1, Q0, S1, S2, SA, Vv, sc):
    """OUT = IEEE-exact (Vv - min) / span, elementwise [128, CPP]."""
    minb, spanb, rspb, mhib, mlob = (sc["minb"][:, :1], sc["spanb"][:, :1],
                                     sc["rspb"][:, :1], sc["mhib"][:, :1],
                                     sc["mlob"][:, :1])
    nc.vector.tensor_scalar(T1[:], Vv, minb, None, OP.subtract)
    nc.vector.tensor_scalar(Q0[:], T1[:], rspb, None, OP.mult)
    nc.vector.tensor_scalar(S1[:], Q0[:], SPLIT_C, None, OP.mult)
    nc.vector.tensor_tensor(out=S2[:], in0=S1[:], in1=Q0[:], op=OP.subtract)
    nc.vector.tensor_tensor(out=S1[:], in0=S1[:], in1=S2[:], op=OP.subtract)  # q_hi
    nc.vector.tensor_tensor(out=S2[:], in0=Q0[:], in1=S1[:], op=OP.subtract)  # q_lo
    nc.vector.tensor_scalar(OUT[:], Q0[:], spanb, None, OP.mult)              # p
    nc.vector.tensor_scalar(SA[:], S1[:], mhib, None, OP.mult)                # qh*mh
    nc.vector.tensor_tensor(out=SA[:], in0=SA[:], in1=OUT[:], op=OP.subtract)
    nc.vector.tensor_scalar(S1[:], S1[:], mlob, None, OP.mult)                # qh*ml
    nc.vector.tensor_tensor(out=SA[:], in0=SA[:], in1=S1[:], op=OP.add)
    nc.vector.tensor_scalar(S1[:], S2[:], mhib, None, OP.mult)                # ql*mh
    nc.vector.tensor_tensor(out=SA[:], in0=SA[:], in1=S1[:], op=OP.add)
    nc.vector.tensor_scalar(S2[:], S2[:], mlob, None, OP.mult)                # ql*ml
    nc.vector.tensor_tensor(out=SA[:], in0=SA[:], in1=S2[:], op=OP.add)       # e1
    nc.vector.tensor_tensor(out=S1[:], in0=T1[:], in1=OUT[:], op=OP.subtract) # rr
    nc.vector.tensor_tensor(out=S1[:], in0=S1[:], in1=SA[:], op=OP.subtract)  # e
    nc.vector.tensor_scalar(S1[:], S1[:], rspb, None, OP.mult)                # e*r
    nc.vector.tensor_tensor(out=OUT[:], in0=Q0[:], in1=S1[:], op=OP.add)      # q


def _batch(nc, tc, b, pts, img, ident, iota16, ones, n_bisect):
    import contextlib
    with contextlib.ExitStack() as ctx:
        sb = ctx.enter_context(tc.tile_pool(name=f"sb{b}", bufs=1))
        ps_ctx = tc.tile_pool(name=f"ps{b}", bufs=2, space="PSUM")
        ps = ps_ctx.__enter__()

        PT = sb.tile([128, CPP * 3], F32, tag="PT", name="PT")
        nc.sync.dma_start(out=PT[:], in_=pts[b].rearrange("(p n) c -> p (n c)", p=128))
        PT3 = PT[:].rearrange("p (n c) -> p c n", c=3)
        Xv, Yv, Zv = PT3[:, 0, :], PT3[:, 1, :], PT3[:, 2, :]

        # f32 scratch tiles shared across phases
        D = sb.tile([128, CPP], F32, tag="D", name="D")
        M = sb.tile([128, CPP], F32, tag="M", name="M")
        VT = sb.tile([128, CPP], F32, tag="VT", name="VT")
        Q0 = sb.tile([128, CPP], F32, tag="Q0", name="Q0")
        S1 = sb.tile([128, CPP], F32, tag="S1", name="S1")
        S2 = sb.tile([128, CPP], F32, tag="S2", name="S2")
        TI = sb.tile([128, CPP], I32, tag="TI", name="TI")

        # ---- dist (exact division) ----
        scz = _scalar_prep(nc, sb, ps, Zv, ident, "z")
        _dekker_div(nc, D, M, Q0, S1, S2, VT, Zv, scz)

        # ---- bisection for q01 ----
        lo = sb.tile([1, 1], F32, tag="lo", name="lo")
        hi = sb.tile([1, 1], F32, tag="hi", name="hi")
        nc.vector.memset(lo[:], 0.0)
        nc.vector.memset(hi[:], 0.0625)
        cntp = sb.tile([128, 1], F32, tag="cntp", name="cntp")
        for it in range(n_bisect):
            mid = sb.tile([1, 1], F32, tag="mid", name="mid")
            nc.vector.tensor_scalar(mid[:], lo[:], hi[:, :1], 0.5, OP.add, OP.mult)
            midb = _bcast(nc, sb, mid, "midb")
            nc.vector.tensor_scalar(M[:], D[:], midb[:, :1], None, OP.is_lt)
            nc.vector.tensor_reduce(cntp[:], M[:], axis=AX.X, op=OP.add)
            tot = ps.tile([1, 1], F32, tag="tot", space="PSUM", name="tot")
            nc.tensor.matmul(tot[:], lhsT=cntp[:, :1], rhs=ones[:, :1], start=True, stop=True)
            pred = sb.tile([1, 1], U8, tag="pred", name="pred")
            nc.vector.tensor_scalar(pred[:], tot[:1, :1], float(QRANK + 1), None, OP.is_ge)
            npred = sb.tile([1, 1], U8, tag="npred", name="npred")
            nc.vector.tensor_scalar(npred[:], tot[:1, :1], float(QRANK + 1), None, OP.is_lt)
            nc.vector.copy_predicated(hi[:], pred[:], mid[:])
            nc.vector.copy_predicated(lo[:], npred[:], mid[:])
        hib = _bcast(nc, sb, hi, "midb")
        nc.vector.tensor_scalar(M[:], D[:], hib[:, :1], None, OP.is_lt)
        nc.vector.tensor_tensor(out=VT[:], in0=M[:], in1=D[:], op=OP.mult)
        nc.vector.tensor_reduce(cntp[:], VT[:], axis=AX.X, op=OP.max)
        vA = _cross(nc, sb, ps, cntp, OP.max, ident, "vA")
        M8 = sb.tile([128, CPP], U8, tag="M8", name="M8")
        nc.vector.tensor_scalar(M8[:], D[:], hib[:, :1], None, OP.is_ge)
        nc.vector.memset(VT[:], 2.0)
        nc.vector.copy_predicated(VT[:], M8[:], D[:])
        nc.vector.tensor_reduce(cntp[:], VT[:], axis=AX.X, op=OP.min)
        vB = _cross(nc, sb, ps, cntp, OP.min, ident, "vB")
        dq = sb.tile([1, 1], F32, tag="dq", name="dq")
        nc.vector.tensor_tensor(out=dq[:], in0=vB[:], in1=vA[:], op=OP.subtract)
        q01 = sb.tile([1, 1], F32, tag="q01", name="q01")
        nc.vector.tensor_scalar(q01[:], dq[:], QFRAC, vA[:, :1], OP.mult, OP.add)
        q01b = _bcast(nc, sb, q01, "q01b")

        # ---- strengths ----
        Wt = sb.tile([128, CPP], F32, tag="Wt", name="Wt")
        nc.vector.tensor_scalar(Wt[:], D[:], q01b[:, :1], None, OP.max)
        nc.vector.tensor_scalar(Wt[:], Wt[:], -1.0, 1.0, OP.mult, OP.add)

        # ---- per-axis prep -> fp16 persistents ----
        def axis_prep(Vv, tag):
            sc = _scalar_prep(nc, sb, ps, Vv, ident, tag)
            IDX = D
            _dekker_div(nc, IDX, M, Q0, S1, S2, VT, Vv, sc)
            nc.vector.tensor_scalar(IDX[:], IDX[:], float(H - 2), 1.0, OP.mult, OP.add)
            # floor via RNE cast + fix (f32 floor into M)
            nc.vector.tensor_copy(TI[:], IDX[:])
            nc.vector.tensor_copy(M[:], TI[:])
            nc.vector.tensor_tensor(out=S1[:], in0=M[:], in1=IDX[:], op=OP.is_gt)
            nc.vector.tensor_tensor(out=M[:], in0=M[:], in1=S1[:], op=OP.subtract)
            F16t = sb.tile([128, CPP], F32, tag=f"F{tag}", name="F16t")
            Fp16t = sb.tile([128, CPP], F32, tag=f"Fp1{tag}", name="Fp16t")
            nc.vector.tensor_copy(F16t[:], M[:])
            nc.vector.tensor_scalar(Fp16t[:], M[:], 1.0, None, OP.add)
            # A = idx - floor (f32 in S2); Ac = (1-A)*(A>0) (f32 in S1)
            nc.vector.tensor_tensor(out=S2[:], in0=IDX[:], in1=M[:], op=OP.subtract)
            nc.vector.tensor_scalar(S1[:], S2[:], -1.0, 1.0, OP.mult, OP.add)
            nc.vector.tensor_scalar(Q0[:], S2[:], 0.0, 1.0, OP.is_gt, OP.mult)
            nc.vector.tensor_tensor(out=S1[:], in0=S1[:], in1=Q0[:], op=OP.mult)
            return F16t, Fp16t

        FX, FXp1 = axis_prep(Xv, "x")
        AXf = sb.tile([128, CPP], F32, tag="AXf", name="AXf")
        AXc = sb.tile([128, CPP], F32, tag="AXc", name="AXc")
        nc.vector.tensor_copy(AXf[:], S2[:])
        nc.vector.tensor_copy(AXc[:], S1[:])

        FY, FYp1 = axis_prep(Yv, "y")
        P1 = sb.tile([128, CPP], F32, tag="P1", name="P1")
        P2 = sb.tile([128, CPP], F32, tag="P2", name="P2")
        nc.vector.tensor_tensor(out=P1[:], in0=S2[:], in1=Wt[:], op=OP.mult)
        nc.vector.tensor_tensor(out=P2[:], in0=S1[:], in1=Wt[:], op=OP.mult)

        ps_ctx.__exit__(None, None, None)

        # ---- binning ----
        for h in range(2):
            with tc.tile_pool(name=f"bps{b}_{h}", bufs=1, space="PSUM") as bps, \
                 tc.tile_pool(name=f"buv{b}_{h}", bufs=4) as uv:
                acc = [bps.tile([128, 512], F32, tag=f"acc{t}", space="PSUM",
                                name=f"acc{t}")
                       for t in range(8)]
                iotaH = iota16[:, h * 512:(h + 1) * 512]

                def chunk(c, start, stop):
                    UV = uv.tile([128, 1536], F16, tag="UV", name="UV")
                    UV2 = uv.tile([128, 1536], F16, tag="UV2", name="UV2")
                    U = UV[:, :512]; V = UV[:, 512:]
                    if isinstance(c, int):
                        sl = (slice(None), slice(c, c + 1))
                    else:
                        sl = (slice(None), bass.DynSlice(c, 1))
                    nc.vector.tensor_scalar(U, iotaH, FX[sl], AXf[sl], OP.is_equal, OP.mult)
                    nc.vector.tensor_scalar(UV2[:, :512], iotaH, FXp1[sl], AXc[sl], OP.is_equal, OP.mult)
                    nc.vector.tensor_scalar(V, iota16[:], FY[sl], P1[sl], OP.is_equal, OP.mult)
                    nc.vector.tensor_scalar(UV2[:, 512:], iota16[:], FYp1[sl], P2[sl], OP.is_equal, OP.mult)
                    nc.vector.tensor_tensor(out=UV[:], in0=UV[:], in1=UV2[:], op=OP.add)
                    for t in range(4):
                        lhs = UV[:, t * 128:(t + 1) * 128]
                        nc.tensor.matmul(acc[2 * t][:], lhsT=lhs, rhs=UV[:, 512:1024],
                                         start=start, stop=stop, skip_group_check=True)
                        nc.tensor.matmul(acc[2 * t + 1][:], lhsT=lhs, rhs=UV[:, 1024:],
                                         start=start, stop=stop, skip_group_check=True)

                chunk(0, True, False)
                tc.For_i_unrolled(1, CPP - 1, 1, lambda iv: chunk(iv, False, False),
                                  max_unroll=24)
                chunk(CPP - 1, False, True)

                for t in range(4):
                    OT = uv.tile([128, W], F32, tag="OT", name="OT")
                    nc.vector.tensor_copy(OT[:, :512], acc[2 * t][:])
                    nc.vector.tensor_copy(OT[:, 512:], acc[2 * t + 1][:])
                    r0 = (h * 4 + t) * 128
                    nc.sync.dma_start(out=img[b, r0:r0 + 128, :], in_=OT[:])


def build_program(num_devices=8, n_bisect=N_BISECT, n_batches=NB):
    nc = bacc.Bacc("TRN2", target_bir_lowering=False, debug=False,
                   num_devices=num_devices)
    pts = nc.dram_tensor("pts", [NB, NPT, 3], F32, kind="ExternalInput")
    img = nc.dram_tensor("img", [NB, H, W], F32, kind="ExternalOutput")

    with tile.TileContext(nc) as tc:
        with tc.tile_pool(name="const", bufs=1) as cp:
            ident = cp.tile([128, 128], F32)
            make_identity(nc, ident[:])
            iota_i = cp.tile([128, W], I32)
            nc.gpsimd.iota(iota_i[:], pattern=[[1, W]], base=0, channel_multiplier=0)
            iota16 = cp.tile([128, W], F16)
            nc.vector.tensor_copy(iota16[:], iota_i[:])
            ones = cp.tile([128, 1], F32)
            nc.vector.memset(ones[:], 1.0)

            for b in range(n_batches):
                _batch(nc, tc, b, pts, img, ident, iota16, ones, n_bisect)
    nc.compile()
    return nc


_NC_CACHE = {}


def get_program():
    if "nc" not in _NC_CACHE:
        _NC_CACHE["nc"] = build_program()
    return _NC_CACHE["nc"]


def kernel(point_clouds: np.ndarray) -> np.ndarray:
    nc = get_program()
    shards = np.ascontiguousarray(point_clouds).reshape(8, NB, NPT, 3)
    in_maps = [{"pts": np.ascontiguousarray(shards[i])} for i in range(8)]
    res = bass_utils.run_bass_kernel_spmd(nc, in_maps, core_ids=list(range(8)))
    out = np.stack([r["img"] for r in res.results])
    return out.reshape(32, H, W)

